# revision 1
# baseline (speedup 1.0000x reference)
"""Trainium2 kernel for nn_AEDecoder: out = LeakyReLU(X @ W_sparse + bias).

The sparse edge list (400k edges over a [1639, 17000] weight matrix, 1.4%
dense) is converted on the host to a dense bf16 weight matrix — the layout the
TensorEngine consumes — with the bias folded in as an extra ones-row of X.
Each of the 8 NeuronCores gets a 2125-gene column shard of W (data-parallel
over output genes, X replicated), runs a tiled bf16 matmul with f32 PSUM
accumulation and a fused LeakyReLU epilogue, and the host concatenates the
per-core [512, 2125] outputs.
"""

import sys

import numpy as np

for _p in ("/opt/trn_rl_repo", "/root/.axon_site/_ro/trn_rl_repo"):
    if _p not in sys.path:
        sys.path.append(_p)

import ml_dtypes

B, IN_F, OUT_F = 512, 1639, 17000
NCORES = 8
SHARD = OUT_F // NCORES      # 2125 output genes per core
K_PAD = 1664                 # 13 * 128 (1639 TF rows + 1 bias row + zero pad)
KC = K_PAD // 128            # 13 contraction chunks
NEG_SLOPE = 0.01
NTILE = 512                  # PSUM bank width in f32

_cache: dict = {}


def _n_tiles():
    tiles = []
    n0 = 0
    while n0 < SHARD:
        tiles.append((n0, min(NTILE, SHARD - n0)))
        n0 += NTILE
    return tiles


def _build_nc():
    import concourse.tile as tile
    from concourse import bacc, mybir

    nc = bacc.Bacc(
        "TRN2",
        target_bir_lowering=False,
        debug=False,
        num_devices=NCORES,
    )
    xT = nc.dram_tensor("xT", [K_PAD, B], mybir.dt.bfloat16, kind="ExternalInput").ap()
    w = nc.dram_tensor("w", [K_PAD, SHARD], mybir.dt.bfloat16, kind="ExternalInput").ap()
    out = nc.dram_tensor("out", [B, SHARD], mybir.dt.float32, kind="ExternalOutput").ap()

    bf16 = mybir.dt.bfloat16
    f32 = mybir.dt.float32

    with tile.TileContext(nc) as tc:
        with (
            tc.tile_pool(name="xp", bufs=1) as xp,
            tc.tile_pool(name="wp", bufs=1) as wp,
            tc.tile_pool(name="pp", bufs=6, space="PSUM") as pp,
            tc.tile_pool(name="op", bufs=4) as op,
        ):
            xts = []
            wts = []
            for k in range(KC):
                xt = xp.tile([128, B], bf16, tag=f"x{k}")
                nc.sync.dma_start(xt[:], xT[k * 128 : (k + 1) * 128, :])
                xts.append(xt)
            for k in range(KC):
                wt = wp.tile([128, SHARD], bf16, tag=f"w{k}")
                nc.sync.dma_start(wt[:], w[k * 128 : (k + 1) * 128, :])
                wts.append(wt)

            for m in range(B // 128):
                for n0, nsz in _n_tiles():
                    pt = pp.tile([128, nsz], f32, tag="psum")
                    for k in range(KC):
                        nc.tensor.matmul(
                            pt[:],
                            lhsT=xts[k][:, m * 128 : (m + 1) * 128],
                            rhs=wts[k][:, n0 : n0 + nsz],
                            start=(k == 0),
                            stop=(k == KC - 1),
                        )
                    ot = op.tile([128, nsz], f32, tag="o")
                    nc.scalar.activation(
                        ot[:],
                        pt[:],
                        mybir.ActivationFunctionType.Lrelu,
                        alpha=NEG_SLOPE,
                    )
                    nc.sync.dma_start(out[m * 128 : (m + 1) * 128, n0 : n0 + nsz], ot[:])

    nc.compile()
    return nc


def kernel(features, weights, bias, edge_out, edge_in):
    from concourse import bass_utils

    features = np.asarray(features, dtype=np.float32)
    weights = np.asarray(weights, dtype=np.float32)
    bias = np.asarray(bias, dtype=np.float32)
    ei = np.asarray(edge_in).astype(np.int64)
    eo = np.asarray(edge_out).astype(np.int64)

    # Sparse edge list -> dense [K_PAD, OUT_F] weight matrix, bias as row IN_F.
    W = np.zeros((K_PAD, OUT_F), dtype=np.float32)
    np.add.at(W, (ei, eo), weights)
    W[IN_F, :] = bias

    xT = np.zeros((K_PAD, B), dtype=np.float32)
    xT[:IN_F] = features.T
    xT[IN_F] = 1.0

    Wb = W.astype(ml_dtypes.bfloat16)
    xTb = np.ascontiguousarray(xT.astype(ml_dtypes.bfloat16))

    if "nc" not in _cache:
        _cache["nc"] = _build_nc()
    nc = _cache["nc"]

    in_maps = [
        {
            "xT": xTb,
            "w": np.ascontiguousarray(Wb[:, c * SHARD : (c + 1) * SHARD]),
        }
        for c in range(NCORES)
    ]
    res = bass_utils.run_bass_kernel_spmd(nc, in_maps, core_ids=list(range(NCORES)))
    return np.concatenate([res.results[c]["out"] for c in range(NCORES)], axis=1)
